# revision 32
# baseline (speedup 1.0000x reference)
"""Bass/Trainium2 kernel for nn_LogRatio loss, data-parallel over anchor rows on 8 cores.

Math: loss = sum_{m,j,k,l} pos[j,k] * N_m[j,l] * (A[j,k] - A[j,l] - c_m)^2
with A = log(X @ X.T + eps). All masks depend on labels only through the
anchor's class t_j (24 classes), so every masked row-reduction is a matmul
against a 0/1 class table W[l, col] (120 cols: [E0, W0..W3]), followed by a
per-row column pick done on the host.

Device per core (256 anchor cols j, all 2048 rows l; l-chunks processed as
[0], (1,2), (3,4) ... (13,14), [15] — the single first chunk lets the Ln
chain start as soon as 128 xt columns have landed, the single last chunk
halves the final act->mul->G2 tail):
  sim    = xt_unit^T @ xjt                    [128 l, 256-512 j] (PE, bf16)
  ash    = ln(sim * e^-A0 + eps * e^-A0)      = A - A0, bf16   (Scalar, one op/unit)
  a2     = ash * ash                          bf16             (Vector, one op/unit)
  G1/G2 += wt_c^T @ ash_c / a2_c  (c in unit)   [128, 256] f32 (PE, PSUM accum)
The A0 shift keeps bf16 rounding of A tiny (A in [2.9, 4.1]); the shift is
undone exactly on the host via the (label-only) table column sums. The class
table is 0/1-valued so fp8e4m3 stores it exactly. Everything after G —
class-column selection, diagonal correction, the m-level combine — is O(N)
and runs on the host in float64.

Scheduling: zero-matmul warmups ramp the PE p-state during the input DMA
window; input DMAs are spread across the SP/ACT/Pool queues so descriptor
generation overlaps; the output writeback descriptors are prepped early and
fired by trigger_dma right after the PSUM->SBUF copies, skipping the
~1.7us tail DMA config chain.
"""

import numpy as np
import ml_dtypes

N, D, KK, C = 2048, 128, 4, 24
NCORES = 8
JPC = N // NCORES          # 256 anchor cols per core
NCH = N // 128             # 16 l-chunks
NPAIR = NCH // 2
NW = 5 * C                 # 120 table columns: [E0, W0, W1, W2, W3]
NWP = 128                  # padded to 128 (kv_writeback wants d_head % 128 == 0)
EPS = 1e-6
OMEGA = 0.1
A0 = 3.5
SCALE = float(np.exp(-A0))

XWCOLS = JPC + N           # bf16 input: [xjt (256) | xt (2048)]

_cache: dict = {}


def _build(repeats: int, use_trigger: bool = True):
    import concourse.bacc as bacc
    import concourse.mybir as mybir
    import concourse.tile as tile

    f32 = mybir.dt.float32
    bf16 = mybir.dt.bfloat16
    fp8 = mybir.dt.float8e4
    AF = mybir.ActivationFunctionType

    nc = bacc.Bacc("TRN2", target_bir_lowering=False, debug=False)
    xw_d = nc.dram_tensor("xw", [128, XWCOLS], bf16, kind="ExternalInput")
    wt_d = nc.dram_tensor("wt", [128, NCH * NWP], fp8, kind="ExternalInput")
    g_d = nc.dram_tensor("g", [1, NWP, 1, 2 * JPC], bf16, kind="ExternalOutput")

    NWARM = 8
    use_trigger = use_trigger and repeats == 1

    with tile.TileContext(nc) as tc:
        with (
            tc.tile_pool(name="const", bufs=1) as const,
            tc.tile_pool(name="work", bufs=4) as work,
            tc.tile_pool(name="gp", bufs=1, space="PSUM") as gp,
            tc.tile_pool(name="psim", bufs=4, space="PSUM") as psim,
            tc.tile_pool(name="wj", bufs=1, space="PSUM") as wj,
        ):
            def body():
                # PE p-state warmup: zero matmuls keep the tensor engine
                # continuously busy from ~1us until real data lands, so the
                # p-state ramp (3us of continuous execution) completes early.
                zr = const.tile([128, 258], bf16, tag="zr")
                nc.vector.memset(zr[:], 0.0)
                epsb = const.tile([128, 1], f32, tag="epsb")
                nc.vector.memset(epsb[:], EPS * SCALE)
                junk = wj.tile([2, JPC], f32, tag="junk")
                for _ in range(NWARM):
                    nc.tensor.matmul(junk[:], zr[:, 256:258], zr[:, 0:256],
                                     start=True, stop=True)

                xw = const.tile([128, XWCOLS], bf16, tag="xw")
                wt = const.tile([128, NCH * NWP], fp8, tag="wt")
                # 4 queues: SP configures fastest -> it carries the first
                # piece (xjt + xt chunks 0-3). Pool takes the class table
                # (needed by the G accumulation), ACT/Pool the rest of xt.
                nc.sync.dma_start(xw[:, 0:JPC + 384], xw_d[:, 0:JPC + 384])
                nc.scalar.dma_start(xw[:, JPC + 384:JPC + 1152],
                                    xw_d[:, JPC + 384:JPC + 1152])
                nc.gpsimd.dma_start(wt[:, 0:8 * NWP], wt_d[:, 0:8 * NWP])
                nc.gpsimd.dma_start(xw[:, JPC + 1152:JPC + 1920],
                                    xw_d[:, JPC + 1152:JPC + 1920])
                nc.sync.dma_start(wt[:, 8 * NWP:NCH * NWP], wt_d[:, 8 * NWP:NCH * NWP])
                nc.sync.dma_start(xw[:, JPC + 1920:XWCOLS],
                                  xw_d[:, JPC + 1920:XWCOLS])
                xjt = xw[:, 0:JPC]

                g1 = gp.tile([NWP, JPC], f32, tag="g1", name="g1")
                g2 = gp.tile([NWP, JPC], f32, tag="g2", name="g2")
                # output writeback: descriptors prepped early, fired by
                # trigger_dma after the PSUM->SBUF copies land
                gsb = work.tile([NWP, 2 * JPC], bf16, tag="gsb")
                if use_trigger:
                    zidx = const.tile([128, 1], mybir.dt.int32, tag="zidx")
                    nc.vector.memset(zidx[:], 0)
                    dma_sem = nc.alloc_semaphore("g_wb")
                    nc.gpsimd.kv_writeback(
                        g_d[:], gsb[:].rearrange("p (a b c) -> p a b c", a=1, b=1),
                        zidx[:], prepare_only=True, sem=dma_sem)

                def sims(unit):
                    sim = psim.tile([128, len(unit) * JPC], f32, tag="sim")
                    for k, ch in enumerate(unit):
                        nc.tensor.matmul(sim[:, k * JPC:(k + 1) * JPC],
                                         xw[:, JPC + 128 * ch:JPC + 128 * (ch + 1)],
                                         xjt, start=True, stop=True)
                    return sim

                # chunk 0 and 15 run as singles: the first act starts as soon
                # as one 128-col chunk of xt has landed, and the last act/mul
                # are half-size, shortening the tail chain.
                units = [[0]] + [[2 * i + 1, 2 * i + 2] for i in range(7)] + [[15]]
                sim = sims(units[0])
                for u, unit in enumerate(units):
                    w = len(unit) * JPC
                    # r layout: [ash_0 (| ash_1) | a2_0 (| a2_1)]
                    r = work.tile([128, 4 * JPC], bf16, tag="r")
                    nc.scalar.activation(r[:, 0:w], sim[:], AF.Ln,
                                         bias=epsb[:], scale=SCALE)
                    if u == len(units) - 1 and len(unit) == 2:
                        nc.vector.tensor_mul(r[:, w:w + JPC],
                                             r[:, 0:JPC], r[:, 0:JPC])
                        nc.vector.tensor_mul(r[:, w + JPC:2 * w],
                                             r[:, JPC:2 * JPC], r[:, JPC:2 * JPC])
                    else:
                        nc.vector.tensor_mul(r[:, w:2 * w], r[:, 0:w], r[:, 0:w])
                    if u + 1 < len(units):
                        sim = sims(units[u + 1])
                    for k, ch in enumerate(unit):
                        # G1 += wt_ch^T @ ash_k  (ready right after the act)
                        nc.tensor.matmul(g1[:], wt[:, NWP * ch:NWP * (ch + 1)],
                                         r[:, k * JPC:(k + 1) * JPC],
                                         start=(ch == 0), stop=(ch == NCH - 1))
                    for k, ch in enumerate(unit):
                        # G2 += wt_ch^T @ a2_k
                        nc.tensor.matmul(g2[:], wt[:, NWP * ch:NWP * (ch + 1)],
                                         r[:, w + k * JPC:w + (k + 1) * JPC],
                                         start=(ch == 0), stop=(ch == NCH - 1))
                # g1 copy on the (idle) scalar engine, g2 on DVE: they run
                # in parallel instead of serializing behind the last mul
                nc.scalar.copy(gsb[:, 0:JPC], g1[:])
                nc.vector.tensor_copy(gsb[:, JPC:2 * JPC], g2[:])
                if use_trigger:
                    # Pool-side read spanning both copies' ranges: its sem wait
                    # blocks the Pool sequencer until the copies land, so the
                    # trigger right after cannot fire the writeback early (the
                    # deferred-RAW edge does not survive the kv_writeback AP).
                    guard = const.tile([128, 2], bf16, tag="guard")
                    nc.gpsimd.tensor_copy(guard[:], gsb[:, JPC - 1:JPC + 1])
                    nc.gpsimd.trigger_dma(count=None)
                    nc.gpsimd.wait_ge(dma_sem, 16)
                else:
                    nc.gpsimd.dma_start(g_d[:].rearrange("a p b c -> p (a b c)"), gsb[:])

            if repeats == 1:
                body()
            else:
                with tc.For_i(0, repeats, 1):
                    body()

    nc.compile()
    return nc


def _tables(labels: np.ndarray):
    lab = np.asarray(labels).astype(np.int64)
    E = (lab[:, :, None] == np.arange(C)[None, None, :]).astype(np.float32)  # [N,4,C]
    W0 = 1.0 - E[:, 3]
    W1 = E[:, 3] * (1.0 - E[:, 2])
    W2 = E[:, 2] * (1.0 - E[:, 1])
    W3 = E[:, 1] * (1.0 - E[:, 0])
    Wtbl = np.concatenate([E[:, 0], W0, W1, W2, W3], axis=1)  # [N, 120], 0/1
    return lab, E, (W0, W1, W2, W3), Wtbl


def _prep_inputs(inputs: np.ndarray, labels: np.ndarray):
    X = np.asarray(inputs, dtype=np.float32)
    _, _, _, Wtbl = _tables(labels)

    XTb = np.ascontiguousarray(X.T).astype(ml_dtypes.bfloat16)  # [128, 2048]
    base = np.empty((128, XWCOLS), dtype=ml_dtypes.bfloat16)
    base[:, JPC:] = XTb
    # wt chunk i at cols [120*i : 120*(i+1)): [128 l-part, 120]
    Wp = np.zeros((N, NWP), dtype=np.float32)
    Wp[:, 0:NW] = Wtbl
    wt = np.ascontiguousarray(
        Wp.reshape(NCH, 128, NWP).transpose(1, 0, 2).reshape(128, NCH * NWP)
    ).astype(ml_dtypes.float8_e4m3)

    in_maps = []
    for core in range(NCORES):
        xw = base.copy()
        xw[:, 0:JPC] = XTb[:, core * JPC:(core + 1) * JPC]
        in_maps.append({"xw": xw, "wt": wt})
    return in_maps


def _host_epilogue(inputs: np.ndarray, labels: np.ndarray, gs: list):
    X = np.asarray(inputs, dtype=np.float32)
    lab, E, Wlev, Wtbl = _tables(labels)
    t = lab[:, 0]

    gs = [g.reshape(NWP, 2 * JPC)[0:NW] for g in gs]
    G1s = np.concatenate([g[:, 0:JPC] for g in gs], axis=1).astype(np.float64)
    G2s = np.concatenate([g[:, JPC:2 * JPC] for g in gs], axis=1).astype(np.float64)
    colsum120 = Wtbl.sum(axis=0).astype(np.float64)[:, None]
    # undo the A0 shift exactly: G1 = sum W*A, G2 = sum W*A^2
    G1 = G1s + A0 * colsum120
    G2 = G2s + 2.0 * A0 * G1s + A0 * A0 * colsum120

    cm = np.array(
        [0.1 * (np.log(OMEGA + EPS) - np.log(OMEGA ** (KK - m + 1) + EPS)) for m in range(KK)],
        dtype=np.float64,
    )
    cnt0 = E[:, 0].sum(axis=0)
    colsum = np.stack([W.sum(axis=0) for W in Wlev])  # [4, C]
    j = np.arange(N)
    dA = np.log((X.astype(np.float64) ** 2).sum(axis=1) + EPS)
    S1 = G1[t, j] - dA
    S2 = G2[t, j] - dA * dA
    Pn = (cnt0[t] - 1.0).astype(np.float64)
    loss = 0.0
    for m in range(KK):
        T1 = G1[C * (1 + m) + t, j]
        T2 = G2[C * (1 + m) + t, j]
        Nn = colsum[m][t].astype(np.float64)
        loss += np.sum(Nn * (S2 - 2.0 * cm[m] * S1 + cm[m] ** 2 * Pn)
                       - 2.0 * (S1 - cm[m] * Pn) * T1
                       + Pn * T2)
    return np.float32(loss)


def _get_nc(repeats: int = 1):
    key = ("nc", repeats)
    if key not in _cache:
        _cache[key] = _build(repeats)
    return _cache[key]


def _get_nc_plain(repeats: int = 1):
    key = ("nc_plain", repeats)
    if key not in _cache:
        _cache[key] = _build(repeats, use_trigger=False)
    return _cache[key]


def _gs_ok(gs):
    # table columns 120-127 are zero, so G rows 120-127 must be exactly 0;
    # stale bf16 work-pool bytes there mean the triggered writeback raced.
    return all(
        not np.count_nonzero(np.asarray(g, dtype=np.float32).reshape(NWP, 2 * JPC)[NW:])
        for g in gs
    )


def run_on_device(inputs, labels, repeats: int = 1, use_trigger: bool = True):
    from concourse.bass_utils import run_bass_kernel_spmd

    nc = _get_nc(repeats) if use_trigger else _get_nc_plain(repeats)
    in_maps = _prep_inputs(inputs, labels)
    res = run_bass_kernel_spmd(nc, in_maps, list(range(NCORES)))
    gs = [res.results[i]["g"] for i in range(NCORES)]
    return gs


def kernel(inputs, labels):
    gs = run_on_device(inputs, labels, repeats=1)
    if not _gs_ok(gs):
        # triggered-writeback integrity check failed; redo with the plain
        # (non-triggered) output DMA tail
        gs = run_on_device(inputs, labels, repeats=1, use_trigger=False)
    total = _host_epilogue(inputs, labels, gs)
    return (total, 0, 0, 0)
